# revision 2
# baseline (speedup 1.0000x reference)
import sys

sys.path.insert(0, "/opt/trn_rl_repo")

import numpy as np

# Problem dims (hardcoded per spec)
B, T, E, H, V, K = 64, 512, 128, 256, 50000, 20
NCORES = 8
BS = B // NCORES          # 8 batch rows per core
NTOK = BS * T             # 4096 tokens per core
G4 = 4 * H                # 1024 gate width per direction


def _sigmoid(x):
    return 1.0 / (1.0 + np.exp(-x))


def _lstm_scan(xg, Whh, reverse):
    # xg: (B,T,4H) f32, Whh: (4H,H)
    b, t, _ = xg.shape
    h = np.zeros((b, H), np.float32)
    c = np.zeros((b, H), np.float32)
    hs = np.empty((b, t, H), np.float32)
    WhhT = np.ascontiguousarray(Whh.T)
    order = range(t - 1, -1, -1) if reverse else range(t)
    for ti in order:
        g = xg[:, ti, :] + h @ WhhT
        i = _sigmoid(g[:, 0:H])
        f = _sigmoid(g[:, H:2 * H])
        gg = np.tanh(g[:, 2 * H:3 * H])
        o = _sigmoid(g[:, 3 * H:4 * H])
        c = f * c + i * gg
        h = o * np.tanh(c)
        hs[:, ti, :] = h
    return hs


def _viterbi(emissions, mask, start_trans, end_trans, transitions):
    # emissions (B,T,K) f32, mask (B,T) bool
    b, t, k = emissions.shape
    score = start_trans[None, :] + emissions[:, 0, :]          # (B,K)
    hist = np.empty((t - 1, b, k), np.int32)
    for ti in range(1, t):
        cand = score[:, :, None] + transitions[None, :, :] + emissions[:, ti, None, :]
        best = cand.max(axis=1)
        idx = cand.argmax(axis=1).astype(np.int32)             # (B,K)
        m = mask[:, ti]
        score = np.where(m[:, None], best, score)
        hist[ti - 1] = idx
    score = score + end_trans[None, :]
    tag = score.argmax(axis=-1).astype(np.int32)               # (B,)
    tags = np.empty((b, t), np.int32)
    tags[:, t - 1] = tag
    ar = np.arange(b)
    for ti in range(t - 2, -1, -1):
        prev = hist[ti][ar, tag]
        tag = np.where(mask[:, ti + 1], prev, tag)
        tags[:, ti] = tag
    return tags


def _build_nc():
    import concourse.bass as bass
    from concourse import mybir
    from concourse.tile import TileContext

    f32 = mybir.dt.float32
    nc = bass.Bass()
    embT = nc.dram_tensor("embT", (E, NTOK), f32, kind="ExternalInput")
    wT = nc.dram_tensor("wT", (E, 2 * G4), f32, kind="ExternalInput")
    out = nc.dram_tensor("xgT", (2 * G4, NTOK), f32, kind="ExternalOutput")

    with TileContext(nc) as tc:
        with (
            tc.tile_pool(name="emb", bufs=1) as ep,
            tc.tile_pool(name="w", bufs=1) as wp,
            tc.tile_pool(name="sb", bufs=4) as sb,
            tc.tile_pool(name="ps", bufs=4, space="PSUM") as ps,
        ):
            embT_sb = ep.tile([E, NTOK], f32)
            nc.sync.dma_start(embT_sb[:], embT[:])
            w_sb = wp.tile([E, 2 * G4], f32)
            nc.sync.dma_start(w_sb[:], wT[:])
            for g in range(2 * G4 // 128):      # 16 gate chunks
                for n in range(NTOK // 512):    # 8 token passes
                    pt = ps.tile([128, 512], f32)
                    nc.tensor.matmul(
                        pt[:],
                        w_sb[:, g * 128:(g + 1) * 128],
                        embT_sb[:, n * 512:(n + 1) * 512],
                        start=True, stop=True,
                    )
                    ot = sb.tile([128, 512], f32)
                    nc.vector.tensor_copy(ot[:], pt[:])
                    nc.sync.dma_start(out[g * 128:(g + 1) * 128, n * 512:(n + 1) * 512], ot[:])
    return nc


LAST_EXEC_TIME_NS = None


def _device_xg(emb_all, Wih_f, Wih_b):
    """emb_all: (B,T,E) f32. Returns xg_f, xg_b (B,T,4H) each, computed on 8 NeuronCores."""
    import os
    global LAST_EXEC_TIME_NS
    from concourse.bass_utils import run_bass_kernel_spmd

    nc = _build_nc()
    wT = np.ascontiguousarray(
        np.concatenate([Wih_f, Wih_b], axis=0).T.astype(np.float32)  # (E, 2048)
    )
    in_maps = []
    for i in range(NCORES):
        shard = emb_all[i * BS:(i + 1) * BS].reshape(NTOK, E)        # (4096,128)
        in_maps.append({
            "embT": np.ascontiguousarray(shard.T.astype(np.float32)),
            "wT": wT,
        })
    trace = bool(os.environ.get("KERNEL_TRACE"))
    res = run_bass_kernel_spmd(nc, in_maps, core_ids=list(range(NCORES)),
                               trace=trace)
    if trace:
        LAST_EXEC_TIME_NS = res.exec_time_ns
    xg_f = np.empty((B, T, G4), np.float32)
    xg_b = np.empty((B, T, G4), np.float32)
    for i in range(NCORES):
        xgT = res.results[i]["xgT"]                                  # (2048, 4096)
        xg_f[i * BS:(i + 1) * BS] = xgT[:G4].T.reshape(BS, T, G4)
        xg_b[i * BS:(i + 1) * BS] = xgT[G4:].T.reshape(BS, T, G4)
    return xg_f, xg_b


def kernel(x, mask, embedding, Wih_f, Whh_f, b_f, Wih_b, Whh_b, b_b,
           Wout, bout, start_trans, end_trans, transitions):
    x = np.asarray(x)
    mask = np.asarray(mask).astype(bool)
    embedding = np.asarray(embedding, np.float32)
    emb = embedding[np.asarray(x, np.int64)]                         # (B,T,E)

    try:
        xg_f, xg_b = _device_xg(emb, np.asarray(Wih_f, np.float32),
                                np.asarray(Wih_b, np.float32))
    except Exception as e:
        sys.stderr.write(f"[kernel] device path failed ({e!r}); numpy fallback\n")
        ef = emb.reshape(B * T, E)
        xg_f = (ef @ np.asarray(Wih_f, np.float32).T).reshape(B, T, G4)
        xg_b = (ef @ np.asarray(Wih_b, np.float32).T).reshape(B, T, G4)

    xg_f = xg_f + np.asarray(b_f, np.float32)[None, None, :]
    xg_b = xg_b + np.asarray(b_b, np.float32)[None, None, :]

    h_f = _lstm_scan(xg_f, np.asarray(Whh_f, np.float32), reverse=False)
    h_b = _lstm_scan(xg_b, np.asarray(Whh_b, np.float32), reverse=True)
    feats = np.concatenate([h_f, h_b], axis=-1)                      # (B,T,2H)
    emissions = feats.reshape(B * T, 2 * H) @ np.asarray(Wout, np.float32).T
    emissions = emissions.reshape(B, T, K) + np.asarray(bout, np.float32)

    tags = _viterbi(emissions, mask, np.asarray(start_trans, np.float32),
                    np.asarray(end_trans, np.float32),
                    np.asarray(transitions, np.float32))
    return tags.astype(np.int32)

